# revision 1
# baseline (speedup 1.0000x reference)
"""Separable depthwise box filter (r=8, 'same' zero padding) on 8 trn2 cores.

Math: per (n, c) plane P (512x512), out = B @ P @ B where B is the symmetric
banded 512x512 matrix with B[i, j] = 1/(2r+1) for |i - j| <= r.  On the PE
(out = lhsT.T @ rhs):

  pass 1: Zt = matmul(lhsT=P,  rhs=B) = P.T @ B   (vertical filter, transposed)
  pass 2: Y  = matmul(lhsT=Zt, rhs=B) = Z  @ B    (horizontal filter, restored)

Both passes stream only the banded columns of B: the K-chunk of rows
[128a, 128a+128) of B has nonzero columns only in [128a-r, 128a+128+r).
PSUM's per-element has_written bit makes the overlapping column windows
accumulate while fresh columns overwrite, so each (M-chunk, K-chunk) pair is
a single matmul: 560 streamed columns per M-chunk instead of 2048.

Sharding: batch dim (8) across the 8 cores; each core filters its 16 channel
planes independently (no cross-core communication).
"""

import numpy as np

_CACHE = {}

N_CORES = 8
P = 128
H = W = 512
A = H // P  # 4 row-chunks per plane


def _band_windows(r):
    """Nonzero column window [n0, n1) of B rows [128a, 128a+128), per a."""
    return [(max(0, P * a - r), min(W, P * a + P + r)) for a in range(A)]


def _build(r, n_planes):
    import concourse.mybir as mybir
    from concourse import bacc
    from concourse.tile import TileContext

    f32 = mybir.dt.float32
    win = _band_windows(r)

    nc = bacc.Bacc()
    x_d = nc.declare_dram_parameter("x", [n_planes * H, W], f32, isOutput=False)
    b_d = nc.declare_dram_parameter("b", [H, W], f32, isOutput=False)
    y_d = nc.declare_dram_parameter("y", [n_planes * H, W], f32, isOutput=True)

    x_ap = x_d.ap().rearrange("(p a q) n -> p q a n", p=n_planes, q=P)
    y_ap = y_d.ap().rearrange("(p a q) n -> p q a n", p=n_planes, q=P)
    b_ap = b_d.ap().rearrange("(a q) n -> q a n", q=P)

    with TileContext(nc) as tc:
        with (
            tc.tile_pool(name="bmat", bufs=1) as bpool,
            tc.tile_pool(name="xin", bufs=3) as xpool,
            tc.tile_pool(name="zmid", bufs=2) as zpool,
            tc.tile_pool(name="yout", bufs=3) as opool,
            tc.tile_pool(name="ps1", bufs=4, space="PSUM") as ps1,
            tc.tile_pool(name="ps2", bufs=4, space="PSUM") as ps2,
        ):
            bt = bpool.tile([P, A, W], f32)
            xt0 = xpool.tile([P, A, W], f32, name="xt0", tag="xt")
            # Interleave plane-0 x chunks with B chunks on the SP HWDGE ring
            # so the a=0 matmuls can start after ~512 KiB instead of 2 MiB;
            # land the first matmul's operands (B window 0 + x chunk 0) first.
            w0, w1 = win[0]
            nc.sync.dma_start(out=bt[:, 0, w0:w1], in_=b_ap[:, 0, w0:w1])
            nc.sync.dma_start(out=xt0[:, 0, :], in_=x_ap[0, :, 0, :])
            nc.sync.dma_start(out=bt[:, 0, w1:W], in_=b_ap[:, 0, w1:W])
            for a in range(1, A):
                nc.sync.dma_start(out=xt0[:, a, :], in_=x_ap[0, :, a, :])
                nc.sync.dma_start(out=bt[:, a, :], in_=b_ap[:, a, :])

            for p in range(n_planes):
                if p == 0:
                    xt = xt0
                else:
                    xt = xpool.tile([P, A, W], f32, name="xt", tag="xt")
                    nc.sync.dma_start(out=xt[:], in_=x_ap[p])

                zt = zpool.tile([P, A, W], f32)
                for m in range(A):
                    ps = ps1.tile([P, W], f32, name="ps1", tag="ps1")
                    for a in range(A):
                        n0, n1 = win[a]
                        nc.tensor.matmul(
                            ps[:, n0:n1],
                            xt[:, a, m * P : (m + 1) * P],
                            bt[:, a, n0:n1],
                            start=(a == 0),
                            stop=(a == A - 1),
                            skip_group_check=True,
                        )
                    if m == 3:
                        nc.scalar.copy(out=zt[:, m, :], in_=ps[:])
                    else:
                        nc.vector.tensor_copy(out=zt[:, m, :], in_=ps[:])

                ot = opool.tile([P, A, W], f32)
                for m in range(A):
                    ps = ps2.tile([P, W], f32)
                    for a in range(A):
                        n0, n1 = win[a]
                        nc.tensor.matmul(
                            ps[:, n0:n1],
                            zt[:, a, m * P : (m + 1) * P],
                            bt[:, a, n0:n1],
                            start=(a == 0),
                            stop=(a == A - 1),
                            skip_group_check=True,
                        )
                    if m == 0:
                        nc.scalar.copy(out=ot[:, m, :], in_=ps[:])
                    else:
                        nc.vector.tensor_copy(out=ot[:, m, :], in_=ps[:])
                    # per-chunk output DMA on the ACT HWDGE ring (parallel to
                    # input's SP ring) so stores overlap the remaining evacs
                    nc.scalar.dma_start(out=y_ap[p, :, m, :], in_=ot[:, m, :])

    # Drop the preamble's GpSimd memsets of unused const tiles: Q7 memsets
    # cost ~µs each and gate the post-preamble all-engine barrier, delaying
    # kernel start.  Keep any const a later instruction actually reads.
    used = set()
    for bb in nc.main_func.blocks:
        for inst in bb.instructions:
            if type(inst).__name__ == "InstMemset":
                continue
            for ap in list(inst.ins or []) + list(inst.outs or []):
                ref = getattr(ap, "memref", None)
                if ref and str(ref).startswith("const-"):
                    used.add(str(ref))
    entry = nc.main_func.blocks[0]
    dropped = [
        inst
        for inst in entry.instructions
        if type(inst).__name__ == "InstMemset"
        and inst.outs
        and str(getattr(inst.outs[0], "memref", "")).startswith("const-")
        and str(inst.outs[0].memref) not in used
    ]
    for inst in dropped:
        entry.instructions.remove(inst)

    nc.finalize()
    return nc


def _box_matrix(r):
    inv_k = np.float32(1.0 / (2 * r + 1))
    b = np.zeros((H, W), dtype=np.float32)
    for i in range(H):
        b[i, max(0, i - r) : min(W, i + r + 1)] = inv_k
    return b


def kernel(x, r):
    from concourse.bass_utils import run_bass_kernel_spmd

    r = int(r)
    x = np.ascontiguousarray(np.asarray(x, dtype=np.float32))
    n, c, h, w = x.shape
    assert (h, w) == (H, W) and n == N_CORES, (n, c, h, w)

    key = (r, c)
    if key not in _CACHE:
        _CACHE[key] = _build(r, c)
    nc = _CACHE[key]

    b = _box_matrix(r)
    in_maps = [{"x": x[i].reshape(c * H, W), "b": b} for i in range(n)]
    res = run_bass_kernel_spmd(nc, in_maps, core_ids=list(range(N_CORES)))
    return np.stack([res.results[i]["y"].reshape(c, H, W) for i in range(n)])



# revision 3
# speedup vs baseline: 1.4247x; 1.4247x over previous
"""Separable depthwise box filter (r=8, 'same' zero padding) on 8 trn2 cores.

Math: per (n, c) plane P (512x512), out = s^2 * (Bo @ P @ Bo) where Bo is the
symmetric banded 512x512 matrix of ONES with |i - j| <= r and s = 1/(2r+1).
Computing with a band of ones keeps B exact in bf16; the s^2 normalization is
folded into the pass-2 PSUM evacuation copy (fp32 scale, then bf16 cast).

On the PE (out = lhsT.T @ rhs):

  pass 1: Zt = matmul(lhsT=P,  rhs=Bo) = P.T @ Bo  (vertical filter, transposed)
  pass 2: Y  = matmul(lhsT=Zt, rhs=Bo) = Z  @ Bo   (horizontal filter, restored)

Everything on-chip is bf16 (inputs cast on host): fp32 matmuls run as HI/LO
pairs at 2x stream cost and disable fast weight load, so bf16 halves PE time
AND halves HBM traffic.  PSUM accumulates in fp32, so only the band sums --
not the accumulation -- see bf16 rounding.

Both passes stream only the banded columns of Bo: the K-chunk of rows
[128a, 128a+128) of Bo has nonzero columns only in [128a-r, 128a+128+r).
PSUM's per-element has_written bit makes the overlapping column windows
accumulate while fresh columns overwrite, so each (M-chunk, K-chunk) pair is
a single matmul: 560 streamed columns per M-chunk instead of 2048.

Sharding: batch dim (8) across the 8 cores; each core filters its 16 channel
planes independently (no cross-core communication).
"""

import numpy as np

_CACHE = {}

N_CORES = 8
P = 128
H = W = 512
A = H // P  # 4 row-chunks per plane


def _band_windows(r):
    """Nonzero column window [n0, n1) of Bo rows [128a, 128a+128), per a."""
    return [(max(0, P * a - r), min(W, P * a + P + r)) for a in range(A)]


def _build(r, n_planes):
    import concourse.mybir as mybir
    from concourse import bacc
    from concourse.tile import TileContext

    bf16 = mybir.dt.bfloat16
    f32 = mybir.dt.float32
    win = _band_windows(r)
    inv_k2 = float(1.0 / float(2 * r + 1) ** 2)

    nc = bacc.Bacc()
    x_d = nc.declare_dram_parameter("x", [n_planes * H, W], bf16, isOutput=False)
    b_d = nc.declare_dram_parameter("b", [H, W], bf16, isOutput=False)
    y_d = nc.declare_dram_parameter("y", [n_planes * H, W], bf16, isOutput=True)

    x_ap = x_d.ap().rearrange("(p a q) n -> p q a n", p=n_planes, q=P)
    y_ap = y_d.ap().rearrange("(p a q) n -> p q a n", p=n_planes, q=P)
    b_ap = b_d.ap().rearrange("(a q) n -> q a n", q=P)

    with TileContext(nc) as tc:
        with (
            tc.tile_pool(name="bmat", bufs=1) as bpool,
            tc.tile_pool(name="xin", bufs=3) as xpool,
            tc.tile_pool(name="zmid", bufs=2) as zpool,
            tc.tile_pool(name="yout", bufs=3) as opool,
            tc.tile_pool(name="ps1", bufs=4, space="PSUM") as ps1,
            tc.tile_pool(name="ps2", bufs=4, space="PSUM") as ps2,
        ):
            bt = bpool.tile([P, A, W], bf16)
            xt0 = xpool.tile([P, A, W], bf16, name="xt0", tag="xt")
            # Land the first matmul's operands (Bo window 0 + plane 0) first so
            # the PE can start before the full Bo matrix has streamed in.
            w0, w1 = win[0]
            nc.sync.dma_start(out=bt[:, 0, w0:w1], in_=b_ap[:, 0, w0:w1])
            nc.sync.dma_start(out=xt0[:], in_=x_ap[0])
            nc.sync.dma_start(out=bt[:, 0, w1:W], in_=b_ap[:, 0, w1:W])
            for a in range(1, A):
                nc.sync.dma_start(out=bt[:, a, :], in_=b_ap[:, a, :])

            for p in range(n_planes):
                if p == 0:
                    xt = xt0
                else:
                    xt = xpool.tile([P, A, W], bf16, name="xt", tag="xt")
                    nc.sync.dma_start(out=xt[:], in_=x_ap[p])

                zt = zpool.tile([P, A, W], bf16)
                for m in range(A):
                    ps = ps1.tile([P, W], f32, name="ps1", tag="ps1")
                    for a in range(A):
                        n0, n1 = win[a]
                        nc.tensor.matmul(
                            ps[:, n0:n1],
                            xt[:, a, m * P : (m + 1) * P],
                            bt[:, a, n0:n1],
                            start=(a == 0),
                            stop=(a == A - 1),
                            skip_group_check=True,
                        )
                    # Alternate PSUM->SBUF evacuations between DVE and ACT so
                    # neither engine becomes the bottleneck.
                    if m % 2 == 0:
                        nc.vector.tensor_copy(out=zt[:, m, :], in_=ps[:])
                    else:
                        nc.scalar.copy(out=zt[:, m, :], in_=ps[:])

                ot = opool.tile([P, A, W], bf16)
                for m in range(A):
                    ps = ps2.tile([P, W], f32)
                    for a in range(A):
                        n0, n1 = win[a]
                        nc.tensor.matmul(
                            ps[:, n0:n1],
                            zt[:, a, m * P : (m + 1) * P],
                            bt[:, a, n0:n1],
                            start=(a == 0),
                            stop=(a == A - 1),
                            skip_group_check=True,
                        )
                    # pass-2 evac folds in the 1/(2r+1)^2 normalization
                    if m % 2 == 0:
                        nc.vector.tensor_scalar_mul(ot[:, m, :], ps[:], inv_k2)
                    else:
                        nc.scalar.mul(ot[:, m, :], ps[:], inv_k2)
                    # per-chunk output DMA on the ACT HWDGE ring (parallel to
                    # input's SP ring) so stores overlap the remaining evacs
                    nc.scalar.dma_start(out=y_ap[p, :, m, :], in_=ot[:, m, :])

    # Drop the preamble's GpSimd memsets of unused const tiles: Q7 memsets
    # cost ~µs each and gate the post-preamble all-engine barrier, delaying
    # kernel start.  Keep any const a later instruction actually reads.
    used = set()
    for bb in nc.main_func.blocks:
        for inst in bb.instructions:
            if type(inst).__name__ == "InstMemset":
                continue
            for ap in list(inst.ins or []) + list(inst.outs or []):
                ref = getattr(ap, "memref", None)
                if ref and str(ref).startswith("const-"):
                    used.add(str(ref))
    entry = nc.main_func.blocks[0]
    dropped = [
        inst
        for inst in entry.instructions
        if type(inst).__name__ == "InstMemset"
        and inst.outs
        and str(getattr(inst.outs[0], "memref", "")).startswith("const-")
        and str(inst.outs[0].memref) not in used
    ]
    for inst in dropped:
        entry.instructions.remove(inst)

    nc.finalize()
    return nc


def _band_ones(r):
    b = np.zeros((H, W), dtype=np.float32)
    for i in range(H):
        b[i, max(0, i - r) : min(W, i + r + 1)] = 1.0
    return b


def kernel(x, r):
    import ml_dtypes
    from concourse.bass_utils import run_bass_kernel_spmd

    r = int(r)
    x = np.asarray(x)
    n, c, h, w = x.shape
    assert (h, w) == (H, W) and n == N_CORES, (n, c, h, w)

    key = (r, c)
    if key not in _CACHE:
        _CACHE[key] = _build(r, c)
    nc = _CACHE[key]

    bf16 = ml_dtypes.bfloat16
    xb = np.ascontiguousarray(x.reshape(n, c * H, W)).astype(bf16)
    b = _band_ones(r).astype(bf16)
    in_maps = [{"x": xb[i], "b": b} for i in range(n)]
    res = run_bass_kernel_spmd(nc, in_maps, core_ids=list(range(N_CORES)))
    out = np.stack(
        [np.asarray(res.results[i]["y"]).astype(np.float32).reshape(c, H, W) for i in range(n)]
    )
    return out


# revision 5
# speedup vs baseline: 1.9905x; 1.3971x over previous
"""Separable depthwise box filter (r=8, 'same' zero padding) on 8 trn2 cores.

Math: per (n, c) plane P (512x512), out = s^2 * (Bo @ P @ Bo) where Bo is the
symmetric banded 512x512 matrix of ONES with |i - j| <= r and s = 1/(2r+1).
Computing with a band of ones keeps B exact in bf16; the s^2 normalization is
folded into the pass-2 PSUM evacuation copy (fp32 scale, then bf16 cast).

On the PE (out = lhsT.T @ rhs):

  pass 1: Zt = matmul(lhsT=P,  rhs=Bo) = P.T @ Bo  (vertical filter, transposed)
  pass 2: Y  = matmul(lhsT=Zt, rhs=Bo) = Z  @ Bo   (horizontal filter, restored)

Everything on-chip is bf16 (inputs cast on host): fp32 matmuls run as HI/LO
pairs at 2x stream cost and disable fast weight load, so bf16 halves PE time
AND halves HBM traffic.  PSUM accumulates in fp32, so only the band sums --
not the accumulation -- see bf16 rounding.

Both passes stream only the banded columns of Bo: the K-chunk of rows
[128a, 128a+128) of Bo has nonzero columns only in [128a-r, 128a+128+r).
PSUM's per-element has_written bit makes the overlapping column windows
accumulate while fresh columns overwrite, so each (M-chunk, K-chunk) pair is
a single matmul: 560 streamed columns per M-chunk instead of 2048.

Sharding: batch dim (8) across the 8 cores; each core filters its 16 channel
planes independently (no cross-core communication).
"""

import numpy as np

_CACHE = {}

N_CORES = 8
P = 128
H = W = 512
A = H // P  # 4 row-chunks per plane


def _band_windows(r):
    """Nonzero column window [n0, n1) of Bo rows [128a, 128a+128), per a."""
    return [(max(0, P * a - r), min(W, P * a + P + r)) for a in range(A)]


def _build(r, n_planes):
    import concourse.mybir as mybir
    from concourse import bacc
    from concourse.tile import TileContext

    bf16 = mybir.dt.bfloat16
    f32 = mybir.dt.float32
    win = _band_windows(r)
    inv_k2 = float(1.0 / float(2 * r + 1) ** 2)

    nc = bacc.Bacc()
    x_d = nc.declare_dram_parameter("x", [n_planes * H, W], bf16, isOutput=False)
    b_d = nc.declare_dram_parameter("b", [H, W], bf16, isOutput=False)
    y_d = nc.declare_dram_parameter("y", [n_planes * H, W], bf16, isOutput=True)

    x_ap = x_d.ap().rearrange("(p a q) n -> p q a n", p=n_planes, q=P)
    y_ap = y_d.ap().rearrange("(p a q) n -> p q a n", p=n_planes, q=P)
    b_ap = b_d.ap().rearrange("(a q) n -> q a n", q=P)

    with TileContext(nc) as tc:
        with (
            tc.tile_pool(name="bmat", bufs=1) as bpool,
            tc.tile_pool(name="xin", bufs=3) as xpool,
            tc.tile_pool(name="zmid", bufs=2) as zpool,
            tc.tile_pool(name="yout", bufs=3) as opool,
            tc.tile_pool(name="ps1", bufs=4, space="PSUM") as ps1,
            tc.tile_pool(name="ps2", bufs=4, space="PSUM") as ps2,
        ):
            bt = bpool.tile([P, A, W], bf16)
            xt0 = xpool.tile([P, A, W], bf16, name="xt0", tag="xt")
            # Land the first matmul's operands (Bo window 0 + plane 0) first so
            # the PE can start before the full Bo matrix has streamed in.
            w0, w1 = win[0]
            nc.sync.dma_start(out=bt[:, 0, w0:w1], in_=b_ap[:, 0, w0:w1])
            nc.sync.dma_start(out=xt0[:], in_=x_ap[0])
            nc.sync.dma_start(out=bt[:, 0, w1:W], in_=b_ap[:, 0, w1:W])
            for a in range(1, A):
                nc.sync.dma_start(out=bt[:, a, :], in_=b_ap[:, a, :])

            for p in range(n_planes):
                if p == 0:
                    xt = xt0
                else:
                    xt = xpool.tile([P, A, W], bf16, name="xt", tag="xt")
                    nc.sync.dma_start(out=xt[:], in_=x_ap[p])

                zt = zpool.tile([P, A, W], bf16)
                for m in range(A):
                    ps = ps1.tile([P, W], f32, name="ps1", tag="ps1")
                    for a in range(A):
                        n0, n1 = win[a]
                        nc.tensor.matmul(
                            ps[:, n0:n1],
                            xt[:, a, m * P : (m + 1) * P],
                            bt[:, a, n0:n1],
                            start=(a == 0),
                            stop=(a == A - 1),
                            skip_group_check=True,
                        )
                    # Alternate PSUM->SBUF evacuations between DVE and ACT so
                    # neither engine becomes the bottleneck.
                    if m % 2 == 0:
                        nc.vector.tensor_copy(out=zt[:, m, :], in_=ps[:])
                    else:
                        nc.scalar.copy(out=zt[:, m, :], in_=ps[:])

                ot = opool.tile([P, A, W], bf16)
                for m in range(A):
                    ps = ps2.tile([P, W], f32)
                    for a in range(A):
                        n0, n1 = win[a]
                        nc.tensor.matmul(
                            ps[:, n0:n1],
                            zt[:, a, m * P : (m + 1) * P],
                            bt[:, a, n0:n1],
                            start=(a == 0),
                            stop=(a == A - 1),
                            skip_group_check=True,
                        )
                    # pass-2 evac folds in the 1/(2r+1)^2 normalization
                    if m % 2 == 0:
                        nc.vector.tensor_scalar_mul(ot[:, m, :], ps[:], inv_k2)
                    else:
                        nc.scalar.mul(ot[:, m, :], ps[:], inv_k2)
                # one store per plane: each dma_start costs the issuing engine
                # ~600ns, so per-chunk stores were burning 39us of ACT time.
                # Issue on SP to keep ACT free for the evacuation copies.
                nc.sync.dma_start(out=y_ap[p], in_=ot[:])

    # Drop the preamble's GpSimd memsets of unused const tiles: Q7 memsets
    # cost ~µs each and gate the post-preamble all-engine barrier, delaying
    # kernel start.  Keep any const a later instruction actually reads.
    used = set()
    for bb in nc.main_func.blocks:
        for inst in bb.instructions:
            if type(inst).__name__ == "InstMemset":
                continue
            for ap in list(inst.ins or []) + list(inst.outs or []):
                ref = getattr(ap, "memref", None)
                if ref and str(ref).startswith("const-"):
                    used.add(str(ref))
    entry = nc.main_func.blocks[0]
    dropped = [
        inst
        for inst in entry.instructions
        if type(inst).__name__ == "InstMemset"
        and inst.outs
        and str(getattr(inst.outs[0], "memref", "")).startswith("const-")
        and str(inst.outs[0].memref) not in used
    ]
    for inst in dropped:
        entry.instructions.remove(inst)

    nc.finalize()
    return nc


def _band_ones(r):
    b = np.zeros((H, W), dtype=np.float32)
    for i in range(H):
        b[i, max(0, i - r) : min(W, i + r + 1)] = 1.0
    return b


def kernel(x, r):
    import ml_dtypes
    from concourse.bass_utils import run_bass_kernel_spmd

    r = int(r)
    x = np.asarray(x)
    n, c, h, w = x.shape
    assert (h, w) == (H, W) and n == N_CORES, (n, c, h, w)

    key = (r, c)
    if key not in _CACHE:
        _CACHE[key] = _build(r, c)
    nc = _CACHE[key]

    bf16 = ml_dtypes.bfloat16
    xb = np.ascontiguousarray(x.reshape(n, c * H, W)).astype(bf16)
    b = _band_ones(r).astype(bf16)
    in_maps = [{"x": xb[i], "b": b} for i in range(n)]
    res = run_bass_kernel_spmd(nc, in_maps, core_ids=list(range(N_CORES)))
    out = np.stack(
        [np.asarray(res.results[i]["y"]).astype(np.float32).reshape(c, H, W) for i in range(n)]
    )
    return out
